# revision 26
# baseline (speedup 1.0000x reference)
"""Trainium2 Bass kernel for CombinedLoss (dice + hausdorff), 8-core SPMD.

Sharding: batch B=32 -> 4 samples/core, 12 (b,c) pairs per core.

Device per (b,c), with fp8(e4m3)-quantized inputs:
    P[rb] = x8 @ y8^T - 0.5*y2[j] - 0.5*x2[i]   (fp8 DoubleRow matmuls,
            rank-1 norm terms appended as a 3-way fp8 split, K=3 matmul)
          = -0.5 * d2[i, j]   for i-block rb, all j          (PSUM f32)
    row path: pm[p, rb] = max_j P[rb]            (DVE tensor_tensor_reduce)
    col path: qm2 = max_rb P -> colv[j] = max_p qm2  (ACT cvt + DVE/Pool max,
              GPSIMD partition reduce)
Host: fp8 quantize + transposes + norms; dice term (exact f32 inputs);
final min/max/sqrt/mean combine from pm ([128,48]) and colv ([1,6144]).
"""

import numpy as np
import ml_dtypes

import concourse.bass as bass
import concourse.bacc as bacc
import concourse.mybir as mybir
import concourse.tile as tile
from concourse.bass_utils import run_bass_kernel_spmd
from bass_rust import AxisListType

B, C, H, W = 32, 3, 512, 512
NCORES = 8
BPC = B // NCORES           # samples per core
NBC = BPC * C               # (b,c) pairs per core = 12
WEIGHT_DICE = 0.4
WEIGHT_HAUSDORFF = 0.6
SMOOTH = 1e-05

f32 = mybir.dt.float32
bf16 = mybir.dt.bfloat16
f8 = mybir.dt.float8e4
ALU = mybir.AluOpType
DR = mybir.MatmulPerfMode.DoubleRow
F8NP = ml_dtypes.float8_e4m3
NEG = -3.0e38

_CACHE = {}


def _build(repeat=1):
    nc = bacc.Bacc(None)
    # x8/y8 transposed and w-interleaved: xyt[bc, w, 0] = x8^T[w], [bc, w, 1] = y8^T[w]
    xyt_d = nc.dram_tensor("xyt", [NBC, W, 2, H], f8, kind="ExternalInput")
    # norm-append operands (3-way fp8 split of -0.5*x2 / -0.5*y2):
    # aug[bc, k, 0:1024] = lhsT (rb, two, m), aug[bc, k, 1024:2048] = rhs (two, n)
    aug_d = nc.dram_tensor("aug", [NBC, 3, 2048], f8, kind="ExternalInput")
    # pm[p, 4*bc+rb] = max_j P[rb]
    # col[bc, 512*rb + j] = max_p P[rb][p, j]  (host folds rb, then min_j)
    pm_d = nc.dram_tensor("pm", [128, 4 * NBC], f32, kind="ExternalOutput")
    col_d = nc.dram_tensor("col", [NBC, 4 * H], f32, kind="ExternalOutput")

    with tile.TileContext(nc) as tc:
        with (
            tc.tile_pool(name="const", bufs=1) as cpool,
            tc.tile_pool(name="xy", bufs=3) as xypool,
            tc.tile_pool(name="aug", bufs=3) as augpool,
            tc.tile_pool(name="q", bufs=2) as qpool,
            tc.tile_pool(name="colp", bufs=6) as colpool,
            tc.tile_pool(name="psum", bufs=2, space="PSUM") as ppool,
        ):
            pmres = cpool.tile([128, 4 * NBC], f32, tag="pmres")

            for bc in [b for _ in range(repeat) for b in range(NBC)]:
                # per-bc norm-append operands; issued on the ACT queue so the
                # SP queue carries only the big input stream
                augt = augpool.tile([3, 2048], f8, tag="augt")
                nc.scalar.dma_start(augt[:], aug_d[bc])
                xyts = xypool.tile([128, 2 * 4 * H], f8, tag="xyts")
                xy5 = xyts[:].rearrange("p (wb t i) -> p wb t i", wb=4, t=2)
                # one input DMA per bc: [p, wb, (t i)], 1KB contiguous runs
                nc.sync.dma_start(
                    xyts[:].rearrange("p (wb ti) -> p wb ti", wb=4),
                    xyt_d[bc].rearrange("(wb p) t i -> p wb (t i)", wb=4),
                )
                al4 = augt[:, 0:1024].rearrange(
                    "k (rb two m) -> k rb two m", rb=4, two=2
                )
                ar = augt[:, 1024:2048].rearrange("k (two n) -> k two n", two=2)

                Ph = [
                    ppool.tile([128, 2 * H], f32, tag=f"P{h}", name=f"P{h}_{bc}")
                    for h in range(2)
                ]
                s01 = qpool.tile([128, 4 * 256], f32, tag="s01")
                coltile = colpool.tile([1, 4 * H], f32, tag="coltile")
                # per rb: 2 main DoubleRow matmuls + norm append, then its row
                # reduce; after each tile pair, its col partition-reduce
                for rb in range(4):
                    h, r = divmod(rb, 2)
                    P = Ph[h][:, 512 * r : 512 * r + 512]
                    for u in range(2):
                        nc.tensor.matmul(
                            P,
                            xy5[:, 2 * u : 2 * u + 2, 0, 128 * rb : 128 * rb + 128],
                            xy5[:, 2 * u : 2 * u + 2, 1, :],
                            start=(u == 0),
                            stop=(u == 1),
                            perf_mode=DR,
                        )
                        if u == 0:
                            # norm append: half0 sums y2 splits, half1 x2 splits
                            nc.tensor.matmul(
                                P, al4[:, rb], ar, start=False, stop=False,
                                perf_mode=DR,
                            )
                    # row path: fused halve-max + j-reduce from PSUM (DVE)
                    nc.vector.tensor_tensor_reduce(
                        out=s01[:, 256 * rb : 256 * rb + 256],
                        in0=Ph[h][:, 512 * r : 512 * r + 256],
                        in1=Ph[h][:, 512 * r + 256 : 512 * r + 512],
                        scale=1.0,
                        scalar=NEG,
                        op0=ALU.max,
                        op1=ALU.max,
                        accum_out=pmres[:, 4 * bc + rb : 4 * bc + rb + 1],
                    )
                    if rb % 2 == 1:
                        # col path: partition max over this tile pair (GPSIMD)
                        nc.gpsimd.tensor_reduce(
                            coltile[:, 1024 * h : 1024 * h + 1024],
                            Ph[h][:],
                            axis=AxisListType.C,
                            op=ALU.max,
                        )
                # ship this bc's col partials while compute continues; issued
                # from the (idle) ACT queue so it can't block input DMAs on SP
                nc.scalar.dma_start(col_d[bc : bc + 1, :], coltile[:])

            nc.sync.dma_start(pm_d[:], pmres[:])
    nc.finalize()
    return nc


def _split3(v):
    """3-term fp8(e4m3) split of v: s1+s2+s3 ~= v to ~2^-12 relative."""
    s1 = v.astype(F8NP)
    r = v - s1.astype(np.float32)
    s2 = r.astype(F8NP)
    r -= s2.astype(np.float32)
    s3 = r.astype(F8NP)
    return s1, s2, s3


def kernel(input, target, _stats=None):
    x = np.asarray(input, dtype=np.float32)
    y = np.asarray(target, dtype=np.float32)

    # ---- host: dice term (exact f32 inputs) ----
    xf = x.reshape(B, -1).astype(np.float64)
    yf = y.reshape(B, -1).astype(np.float64)
    inter = (xf * yf).sum(axis=1)
    union = xf.sum(axis=1) + yf.sum(axis=1)
    dice = float(np.mean(1.0 - (2.0 * inter + SMOOTH) / (union + SMOOTH)))

    # ---- host: fp8 quantize + layout prep ----
    x8 = x.astype(F8NP)
    y8 = y.astype(F8NP)
    xq = x8.astype(np.float64)
    yq = y8.astype(np.float64)
    x2 = (xq * xq).sum(axis=-1).astype(np.float32)  # [B,C,H] norms of quantized pts
    y2 = (yq * yq).sum(axis=-1).astype(np.float32)
    xt8 = np.ascontiguousarray(x8.transpose(0, 1, 3, 2))  # [B,C,W,H] fp8
    yt8 = np.ascontiguousarray(y8.transpose(0, 1, 3, 2))

    in_maps = []
    for c in range(NCORES):
        b0 = c * BPC
        # [NBC, W, 2, H]: x^T and y^T interleaved per w row
        xyt = np.stack(
            [
                xt8[b0 : b0 + BPC].reshape(NBC, W, H),
                yt8[b0 : b0 + BPC].reshape(NBC, W, H),
            ],
            axis=2,
        )
        # lhsT[k, bc, rb, 0, :] = 1 ; [k, bc, rb, 1, :] = k-th split of -0.5*x2
        xs = _split3((-0.5 * x2[b0 : b0 + BPC]).reshape(NBC, 4, 128))
        auglhs = np.zeros((3, NBC, 4, 2, 128), dtype=F8NP)
        auglhs[:, :, :, 0, :] = np.float32(1.0)
        for k in range(3):
            auglhs[k, :, :, 1, :] = xs[k]
        # rhs[k, bc, 0, :] = k-th split of -0.5*y2 ; rhs[k, bc, 1, :] = 1
        ys = _split3((-0.5 * y2[b0 : b0 + BPC]).reshape(NBC, H))
        augrhs = np.zeros((3, NBC, 2, H), dtype=F8NP)
        for k in range(3):
            augrhs[k, :, 0, :] = ys[k]
        augrhs[:, :, 1, :] = np.float32(1.0)
        aug = np.concatenate(
            [auglhs.reshape(3, NBC, 1024), augrhs.reshape(3, NBC, 1024)], axis=2
        ).transpose(1, 0, 2)  # [NBC, 3, 2048]
        in_maps.append(
            {
                "xyt": np.ascontiguousarray(xyt),
                "aug": np.ascontiguousarray(aug),
            }
        )

    if "nc" not in _CACHE:
        _CACHE["nc"] = _build()
    nc = _CACHE["nc"]

    import time as _time

    t0 = _time.time()
    br = run_bass_kernel_spmd(nc, in_maps, list(range(NCORES)), trace=False)
    t1 = _time.time()
    if isinstance(_stats, dict):
        _stats["wall_s"] = t1 - t0
        reps = _stats.get("repeats", 0)
        times = []
        for _ in range(reps):
            ta = _time.time()
            br = run_bass_kernel_spmd(nc, in_maps, list(range(NCORES)), trace=False)
            times.append(_time.time() - ta)
        _stats["repeat_wall_s"] = times

    # ---- host: combine ----
    hds = []
    for c in range(NCORES):
        pm = np.asarray(br.results[c]["pm"], dtype=np.float32)  # [128, 48]
        colv = np.asarray(br.results[c]["col"], dtype=np.float32).reshape(
            NBC, 4, H
        )  # per-bc col partials by i-block
        for bc in range(NBC):
            dxy2 = -2.0 * float(pm[:, 4 * bc : 4 * bc + 4].min())
            dyx2 = -2.0 * float(colv[bc].max(axis=0).min())
            hds.append(np.sqrt(max(dxy2, dyx2, 0.0)))
    hd = float(np.mean(hds))

    loss = WEIGHT_DICE * dice + WEIGHT_HAUSDORFF * hd
    return np.float32(loss)


# revision 27
# speedup vs baseline: 1.0735x; 1.0735x over previous
"""Trainium2 Bass kernel for CombinedLoss (dice + hausdorff), 8-core SPMD.

Sharding: batch B=32 -> 4 samples/core, 12 (b,c) pairs per core.

Device per (b,c), with fp8(e4m3)-quantized inputs:
    P[rb] = x8 @ y8^T - 0.5*y2[j] - 0.5*x2[i]   (fp8 DoubleRow matmuls,
            rank-1 norm terms appended as a 3-way fp8 split, K=3 matmul)
          = -0.5 * d2[i, j]   for i-block rb, all j          (PSUM f32)
    row path: pm[p, rb] = max_j P[rb]            (DVE tensor_tensor_reduce)
    col path: qm2 = max_rb P -> colv[j] = max_p qm2  (ACT cvt + DVE/Pool max,
              GPSIMD partition reduce)
Host: fp8 quantize + transposes + norms; dice term (exact f32 inputs);
final min/max/sqrt/mean combine from pm ([128,48]) and colv ([1,6144]).
"""

import numpy as np
import ml_dtypes

import concourse.bass as bass
import concourse.bacc as bacc
import concourse.mybir as mybir
import concourse.tile as tile
from concourse.bass_utils import run_bass_kernel_spmd
from bass_rust import AxisListType

B, C, H, W = 32, 3, 512, 512
NCORES = 8
BPC = B // NCORES           # samples per core
NBC = BPC * C               # (b,c) pairs per core = 12
WEIGHT_DICE = 0.4
WEIGHT_HAUSDORFF = 0.6
SMOOTH = 1e-05

f32 = mybir.dt.float32
bf16 = mybir.dt.bfloat16
f8 = mybir.dt.float8e4
ALU = mybir.AluOpType
DR = mybir.MatmulPerfMode.DoubleRow
F8NP = ml_dtypes.float8_e4m3
NEG = -3.0e38

_CACHE = {}


def _build(repeat=1):
    nc = bacc.Bacc(None)
    # x8/y8 transposed and w-interleaved: xyt[bc, w, 0] = x8^T[w], [bc, w, 1] = y8^T[w]
    xyt_d = nc.dram_tensor("xyt", [NBC, W, 2, H], f8, kind="ExternalInput")
    # norm-append operands (3-way fp8 split of -0.5*x2 / -0.5*y2):
    # aug[bc, k, 0:1024] = lhsT (rb, two, m), aug[bc, k, 1024:2048] = rhs (two, n)
    aug_d = nc.dram_tensor("aug", [NBC, 3, 2048], f8, kind="ExternalInput")
    # pm[p, 4*bc+rb] = max_j P[rb]
    # col[bc, 512*rb + j] = max_p P[rb][p, j]  (host folds rb, then min_j)
    pm_d = nc.dram_tensor("pm", [128, 4 * NBC], f32, kind="ExternalOutput")
    col_d = nc.dram_tensor("col", [NBC, 4 * H], f32, kind="ExternalOutput")

    with tile.TileContext(nc) as tc:
        with (
            tc.tile_pool(name="const", bufs=1) as cpool,
            tc.tile_pool(name="xy", bufs=3) as xypool,
            tc.tile_pool(name="aug", bufs=3) as augpool,
            tc.tile_pool(name="q", bufs=2) as qpool,
            tc.tile_pool(name="colp", bufs=6) as colpool,
            tc.tile_pool(name="psum", bufs=2, space="PSUM") as ppool,
        ):
            pmres = cpool.tile([128, 4 * NBC], f32, tag="pmres")

            pending_col = []  # (bc, coltile) awaiting shipment (lag-2)
            for bc in [b for _ in range(repeat) for b in range(NBC)]:
                xyts = xypool.tile([128, 2 * 4 * H], f8, tag="xyts")
                xy5 = xyts[:].rearrange("p (wb t i) -> p wb t i", wb=4, t=2)
                xyv = xyts[:].rearrange("p (wb ti) -> p wb ti", wb=4)
                xysrc = xyt_d[bc].rearrange("(wb p) t i -> p wb (t i)", wb=4)
                if bc == 0:
                    # split the first load across both queues to cut the
                    # time-to-first-matmul roughly in half
                    nc.sync.dma_start(xyv[:, 0:2, :], xysrc[:, 0:2, :])
                    nc.scalar.dma_start(xyv[:, 2:4, :], xysrc[:, 2:4, :])
                else:
                    nc.sync.dma_start(xyv, xysrc)
                # per-bc norm-append operands; issued on the ACT queue so the
                # SP queue carries only the big input stream
                augt = augpool.tile([3, 2048], f8, tag="augt")
                nc.scalar.dma_start(augt[:], aug_d[bc])
                # ship col partials from 2 iterations ago (data long since
                # ready -> no head-of-line blocking on the ACT queue)
                if len(pending_col) >= 2:
                    pbc, ptile = pending_col.pop(0)
                    nc.scalar.dma_start(col_d[pbc : pbc + 1, :], ptile[:])
                al4 = augt[:, 0:1024].rearrange(
                    "k (rb two m) -> k rb two m", rb=4, two=2
                )
                ar = augt[:, 1024:2048].rearrange("k (two n) -> k two n", two=2)

                Ph = [
                    ppool.tile([128, 2 * H], f32, tag=f"P{h}", name=f"P{h}_{bc}")
                    for h in range(2)
                ]
                s01 = qpool.tile([128, 4 * 256], f32, tag="s01")
                coltile = colpool.tile([1, 4 * H], f32, tag="coltile")
                # per rb: 2 main DoubleRow matmuls + norm append (last, so the
                # aug DMA is off the critical path); col reduce per tile pair
                # emitted before the row ttrs (readers serialize in emission
                # order; Pool is the scarce engine)
                for h in range(2):
                    for r in range(2):
                        rb = 2 * h + r
                        P = Ph[h][:, 512 * r : 512 * r + 512]
                        for u in range(2):
                            nc.tensor.matmul(
                                P,
                                xy5[:, 2 * u : 2 * u + 2, 0, 128 * rb : 128 * rb + 128],
                                xy5[:, 2 * u : 2 * u + 2, 1, :],
                                start=(u == 0),
                                stop=False,
                                perf_mode=DR,
                            )
                        # norm append: half0 sums y2 splits, half1 x2 splits
                        nc.tensor.matmul(
                            P, al4[:, rb], ar, start=False, stop=True,
                            perf_mode=DR,
                        )
                    # col path: partition max over this tile pair (GPSIMD)
                    nc.gpsimd.tensor_reduce(
                        coltile[:, 1024 * h : 1024 * h + 1024],
                        Ph[h][:],
                        axis=AxisListType.C,
                        op=ALU.max,
                    )
                    # row path: fused halve-max + j-reduce from PSUM (DVE)
                    for r in range(2):
                        rb = 2 * h + r
                        nc.vector.tensor_tensor_reduce(
                            out=s01[:, 256 * rb : 256 * rb + 256],
                            in0=Ph[h][:, 512 * r : 512 * r + 256],
                            in1=Ph[h][:, 512 * r + 256 : 512 * r + 512],
                            scale=1.0,
                            scalar=NEG,
                            op0=ALU.max,
                            op1=ALU.max,
                            accum_out=pmres[:, 4 * bc + rb : 4 * bc + rb + 1],
                        )
                pending_col.append((bc, coltile))

            for pbc, ptile in pending_col:
                nc.scalar.dma_start(col_d[pbc : pbc + 1, :], ptile[:])
            nc.sync.dma_start(pm_d[:], pmres[:])
    nc.finalize()
    return nc


def _split3(v):
    """3-term fp8(e4m3) split of v: s1+s2+s3 ~= v to ~2^-12 relative."""
    s1 = v.astype(F8NP)
    r = v - s1.astype(np.float32)
    s2 = r.astype(F8NP)
    r -= s2.astype(np.float32)
    s3 = r.astype(F8NP)
    return s1, s2, s3


def kernel(input, target, _stats=None):
    x = np.asarray(input, dtype=np.float32)
    y = np.asarray(target, dtype=np.float32)

    # ---- host: dice term (exact f32 inputs) ----
    xf = x.reshape(B, -1).astype(np.float64)
    yf = y.reshape(B, -1).astype(np.float64)
    inter = (xf * yf).sum(axis=1)
    union = xf.sum(axis=1) + yf.sum(axis=1)
    dice = float(np.mean(1.0 - (2.0 * inter + SMOOTH) / (union + SMOOTH)))

    # ---- host: fp8 quantize + layout prep ----
    x8 = x.astype(F8NP)
    y8 = y.astype(F8NP)
    xq = x8.astype(np.float64)
    yq = y8.astype(np.float64)
    x2 = (xq * xq).sum(axis=-1).astype(np.float32)  # [B,C,H] norms of quantized pts
    y2 = (yq * yq).sum(axis=-1).astype(np.float32)
    xt8 = np.ascontiguousarray(x8.transpose(0, 1, 3, 2))  # [B,C,W,H] fp8
    yt8 = np.ascontiguousarray(y8.transpose(0, 1, 3, 2))

    in_maps = []
    for c in range(NCORES):
        b0 = c * BPC
        # [NBC, W, 2, H]: x^T and y^T interleaved per w row
        xyt = np.stack(
            [
                xt8[b0 : b0 + BPC].reshape(NBC, W, H),
                yt8[b0 : b0 + BPC].reshape(NBC, W, H),
            ],
            axis=2,
        )
        # lhsT[k, bc, rb, 0, :] = 1 ; [k, bc, rb, 1, :] = k-th split of -0.5*x2
        xs = _split3((-0.5 * x2[b0 : b0 + BPC]).reshape(NBC, 4, 128))
        auglhs = np.zeros((3, NBC, 4, 2, 128), dtype=F8NP)
        auglhs[:, :, :, 0, :] = np.float32(1.0)
        for k in range(3):
            auglhs[k, :, :, 1, :] = xs[k]
        # rhs[k, bc, 0, :] = k-th split of -0.5*y2 ; rhs[k, bc, 1, :] = 1
        ys = _split3((-0.5 * y2[b0 : b0 + BPC]).reshape(NBC, H))
        augrhs = np.zeros((3, NBC, 2, H), dtype=F8NP)
        for k in range(3):
            augrhs[k, :, 0, :] = ys[k]
        augrhs[:, :, 1, :] = np.float32(1.0)
        aug = np.concatenate(
            [auglhs.reshape(3, NBC, 1024), augrhs.reshape(3, NBC, 1024)], axis=2
        ).transpose(1, 0, 2)  # [NBC, 3, 2048]
        in_maps.append(
            {
                "xyt": np.ascontiguousarray(xyt),
                "aug": np.ascontiguousarray(aug),
            }
        )

    if "nc" not in _CACHE:
        _CACHE["nc"] = _build()
    nc = _CACHE["nc"]

    import time as _time

    t0 = _time.time()
    br = run_bass_kernel_spmd(nc, in_maps, list(range(NCORES)), trace=False)
    t1 = _time.time()
    if isinstance(_stats, dict):
        _stats["wall_s"] = t1 - t0
        reps = _stats.get("repeats", 0)
        times = []
        for _ in range(reps):
            ta = _time.time()
            br = run_bass_kernel_spmd(nc, in_maps, list(range(NCORES)), trace=False)
            times.append(_time.time() - ta)
        _stats["repeat_wall_s"] = times

    # ---- host: combine ----
    hds = []
    for c in range(NCORES):
        pm = np.asarray(br.results[c]["pm"], dtype=np.float32)  # [128, 48]
        colv = np.asarray(br.results[c]["col"], dtype=np.float32).reshape(
            NBC, 4, H
        )  # per-bc col partials by i-block
        for bc in range(NBC):
            dxy2 = -2.0 * float(pm[:, 4 * bc : 4 * bc + 4].min())
            dyx2 = -2.0 * float(colv[bc].max(axis=0).min())
            hds.append(np.sqrt(max(dxy2, dyx2, 0.0)))
    hd = float(np.mean(hds))

    loss = WEIGHT_DICE * dice + WEIGHT_HAUSDORFF * hd
    return np.float32(loss)


# revision 28
# speedup vs baseline: 1.1546x; 1.0755x over previous
"""Trainium2 Bass kernel for CombinedLoss (dice + hausdorff), 8-core SPMD.

Sharding: batch B=32 -> 4 samples/core, 12 (b,c) pairs per core.

Device per (b,c), with fp8(e4m3)-quantized inputs:
    P[rb] = x8 @ y8^T - 0.5*y2[j] - 0.5*x2[i]   (fp8 DoubleRow matmuls,
            rank-1 norm terms appended as a 3-way fp8 split, K=3 matmul)
          = -0.5 * d2[i, j]   for i-block rb, all j          (PSUM f32)
    row path: pm[p, rb] = max_j P[rb]            (DVE tensor_tensor_reduce)
    col path: qm2 = max_rb P -> colv[j] = max_p qm2  (ACT cvt + DVE/Pool max,
              GPSIMD partition reduce)
Host: fp8 quantize + transposes + norms; dice term (exact f32 inputs);
final min/max/sqrt/mean combine from pm ([128,48]) and colv ([1,6144]).
"""

import numpy as np
import ml_dtypes

import concourse.bass as bass
import concourse.bacc as bacc
import concourse.mybir as mybir
import concourse.tile as tile
from concourse.bass_utils import run_bass_kernel_spmd
from bass_rust import AxisListType

B, C, H, W = 32, 3, 512, 512
NCORES = 8
BPC = B // NCORES           # samples per core
NBC = BPC * C               # (b,c) pairs per core = 12
WEIGHT_DICE = 0.4
WEIGHT_HAUSDORFF = 0.6
SMOOTH = 1e-05

f32 = mybir.dt.float32
bf16 = mybir.dt.bfloat16
f8 = mybir.dt.float8e4
ALU = mybir.AluOpType
DR = mybir.MatmulPerfMode.DoubleRow
F8NP = ml_dtypes.float8_e4m3
NEG = -3.0e38

_CACHE = {}


def _build(repeat=1):
    nc = bacc.Bacc(None)
    # x8/y8 transposed and w-interleaved: xyt[bc, w, 0] = x8^T[w], [bc, w, 1] = y8^T[w]
    xyt_d = nc.dram_tensor("xyt", [NBC, W, 2, H], f8, kind="ExternalInput")
    # norm-append operands (3-way fp8 split of -0.5*x2 / -0.5*y2):
    # aug[bc, k, 0:1024] = lhsT (rb, two, m), aug[bc, k, 1024:2048] = rhs (two, n)
    aug_d = nc.dram_tensor("aug", [NBC, 3, 2048], f8, kind="ExternalInput")
    # pm[p, 4*bc+rb] = max_j P[rb]
    # col[bc, 512*rb + j] = max_p P[rb][p, j]  (host folds rb, then min_j)
    pm_d = nc.dram_tensor("pm", [128, 4 * NBC], f32, kind="ExternalOutput")
    col_d = nc.dram_tensor("col", [NBC, 4 * H], f32, kind="ExternalOutput")

    with tile.TileContext(nc) as tc:
        with (
            tc.tile_pool(name="const", bufs=1) as cpool,
            tc.tile_pool(name="xy", bufs=3) as xypool,
            tc.tile_pool(name="aug", bufs=3) as augpool,
            tc.tile_pool(name="q", bufs=2) as qpool,
            tc.tile_pool(name="colp", bufs=6) as colpool,
            tc.tile_pool(name="psum", bufs=2, space="PSUM") as ppool,
        ):
            pmres = cpool.tile([128, 4 * NBC], f32, tag="pmres")

            pending_col = []  # (bc, coltile) awaiting shipment (lag-2)
            for bc in [b for _ in range(repeat) for b in range(NBC)]:
                xyts = xypool.tile([128, 2 * 4 * H], f8, tag="xyts")
                xy5 = xyts[:].rearrange("p (wb t i) -> p wb t i", wb=4, t=2)
                xyv = xyts[:].rearrange("p (wb ti) -> p wb ti", wb=4)
                xysrc = xyt_d[bc].rearrange("(wb p) t i -> p wb (t i)", wb=4)
                if bc == 0:
                    # split the first load across both queues to cut the
                    # time-to-first-matmul roughly in half
                    nc.sync.dma_start(xyv[:, 0:2, :], xysrc[:, 0:2, :])
                    nc.scalar.dma_start(xyv[:, 2:4, :], xysrc[:, 2:4, :])
                else:
                    nc.sync.dma_start(xyv, xysrc)
                # per-bc norm-append operands; issued on the ACT queue so the
                # SP queue carries only the big input stream
                augt = augpool.tile([3, 2048], f8, tag="augt")
                nc.scalar.dma_start(augt[:], aug_d[bc])
                # ship col partials from 2 iterations ago (data long since
                # ready -> no head-of-line blocking on the ACT queue)
                if len(pending_col) >= 2:
                    pbc, ptile = pending_col.pop(0)
                    nc.scalar.dma_start(col_d[pbc : pbc + 1, :], ptile[:])
                al4 = augt[:, 0:1024].rearrange(
                    "k (rb two m) -> k rb two m", rb=4, two=2
                )
                ar = augt[:, 1024:2048].rearrange("k (two n) -> k two n", two=2)

                s01 = qpool.tile([128, 4 * 256], f32, tag="s01")
                coltile = colpool.tile([1, 4 * H], f32, tag="coltile")
                # per rb: one single-bank PSUM tile (fine-grained rotation),
                # 2 main DoubleRow matmuls + norm append, col partition-reduce
                # (Pool, the scarce engine, emitted first), then the row ttr
                for rb in range(4):
                    P = ppool.tile([128, H], f32, tag=f"P{rb}", name=f"P{rb}_{bc}")
                    for u in range(2):
                        nc.tensor.matmul(
                            P[:],
                            xy5[:, 2 * u : 2 * u + 2, 0, 128 * rb : 128 * rb + 128],
                            xy5[:, 2 * u : 2 * u + 2, 1, :],
                            start=(u == 0),
                            stop=False,
                            perf_mode=DR,
                        )
                    # norm append: half0 sums y2 splits, half1 x2 splits
                    nc.tensor.matmul(
                        P[:], al4[:, rb], ar, start=False, stop=True,
                        perf_mode=DR,
                    )
                    # col path: partition max for this tile (GPSIMD)
                    nc.gpsimd.tensor_reduce(
                        coltile[:, 512 * rb : 512 * rb + 512],
                        P[:],
                        axis=AxisListType.C,
                        op=ALU.max,
                    )
                    # row path: fused halve-max + j-reduce from PSUM (DVE)
                    nc.vector.tensor_tensor_reduce(
                        out=s01[:, 256 * rb : 256 * rb + 256],
                        in0=P[:, 0:256],
                        in1=P[:, 256:512],
                        scale=1.0,
                        scalar=NEG,
                        op0=ALU.max,
                        op1=ALU.max,
                        accum_out=pmres[:, 4 * bc + rb : 4 * bc + rb + 1],
                    )
                pending_col.append((bc, coltile))

            for pbc, ptile in pending_col:
                nc.scalar.dma_start(col_d[pbc : pbc + 1, :], ptile[:])
            nc.sync.dma_start(pm_d[:], pmres[:])
    nc.finalize()
    return nc


def _split3(v):
    """3-term fp8(e4m3) split of v: s1+s2+s3 ~= v to ~2^-12 relative."""
    s1 = v.astype(F8NP)
    r = v - s1.astype(np.float32)
    s2 = r.astype(F8NP)
    r -= s2.astype(np.float32)
    s3 = r.astype(F8NP)
    return s1, s2, s3


def kernel(input, target, _stats=None):
    x = np.asarray(input, dtype=np.float32)
    y = np.asarray(target, dtype=np.float32)

    # ---- host: dice term (exact f32 inputs) ----
    xf = x.reshape(B, -1).astype(np.float64)
    yf = y.reshape(B, -1).astype(np.float64)
    inter = (xf * yf).sum(axis=1)
    union = xf.sum(axis=1) + yf.sum(axis=1)
    dice = float(np.mean(1.0 - (2.0 * inter + SMOOTH) / (union + SMOOTH)))

    # ---- host: fp8 quantize + layout prep ----
    x8 = x.astype(F8NP)
    y8 = y.astype(F8NP)
    xq = x8.astype(np.float64)
    yq = y8.astype(np.float64)
    x2 = (xq * xq).sum(axis=-1).astype(np.float32)  # [B,C,H] norms of quantized pts
    y2 = (yq * yq).sum(axis=-1).astype(np.float32)
    xt8 = np.ascontiguousarray(x8.transpose(0, 1, 3, 2))  # [B,C,W,H] fp8
    yt8 = np.ascontiguousarray(y8.transpose(0, 1, 3, 2))

    in_maps = []
    for c in range(NCORES):
        b0 = c * BPC
        # [NBC, W, 2, H]: x^T and y^T interleaved per w row
        xyt = np.stack(
            [
                xt8[b0 : b0 + BPC].reshape(NBC, W, H),
                yt8[b0 : b0 + BPC].reshape(NBC, W, H),
            ],
            axis=2,
        )
        # lhsT[k, bc, rb, 0, :] = 1 ; [k, bc, rb, 1, :] = k-th split of -0.5*x2
        xs = _split3((-0.5 * x2[b0 : b0 + BPC]).reshape(NBC, 4, 128))
        auglhs = np.zeros((3, NBC, 4, 2, 128), dtype=F8NP)
        auglhs[:, :, :, 0, :] = np.float32(1.0)
        for k in range(3):
            auglhs[k, :, :, 1, :] = xs[k]
        # rhs[k, bc, 0, :] = k-th split of -0.5*y2 ; rhs[k, bc, 1, :] = 1
        ys = _split3((-0.5 * y2[b0 : b0 + BPC]).reshape(NBC, H))
        augrhs = np.zeros((3, NBC, 2, H), dtype=F8NP)
        for k in range(3):
            augrhs[k, :, 0, :] = ys[k]
        augrhs[:, :, 1, :] = np.float32(1.0)
        aug = np.concatenate(
            [auglhs.reshape(3, NBC, 1024), augrhs.reshape(3, NBC, 1024)], axis=2
        ).transpose(1, 0, 2)  # [NBC, 3, 2048]
        in_maps.append(
            {
                "xyt": np.ascontiguousarray(xyt),
                "aug": np.ascontiguousarray(aug),
            }
        )

    if "nc" not in _CACHE:
        _CACHE["nc"] = _build()
    nc = _CACHE["nc"]

    import time as _time

    t0 = _time.time()
    br = run_bass_kernel_spmd(nc, in_maps, list(range(NCORES)), trace=False)
    t1 = _time.time()
    if isinstance(_stats, dict):
        _stats["wall_s"] = t1 - t0
        reps = _stats.get("repeats", 0)
        times = []
        for _ in range(reps):
            ta = _time.time()
            br = run_bass_kernel_spmd(nc, in_maps, list(range(NCORES)), trace=False)
            times.append(_time.time() - ta)
        _stats["repeat_wall_s"] = times

    # ---- host: combine ----
    hds = []
    for c in range(NCORES):
        pm = np.asarray(br.results[c]["pm"], dtype=np.float32)  # [128, 48]
        colv = np.asarray(br.results[c]["col"], dtype=np.float32).reshape(
            NBC, 4, H
        )  # per-bc col partials by i-block
        for bc in range(NBC):
            dxy2 = -2.0 * float(pm[:, 4 * bc : 4 * bc + 4].min())
            dyx2 = -2.0 * float(colv[bc].max(axis=0).min())
            hds.append(np.sqrt(max(dxy2, dyx2, 0.0)))
    hd = float(np.mean(hds))

    loss = WEIGHT_DICE * dice + WEIGHT_HAUSDORFF * hd
    return np.float32(loss)


# revision 31
# speedup vs baseline: 1.2290x; 1.0644x over previous
"""Trainium2 Bass kernel for CombinedLoss (dice + hausdorff), 8-core SPMD.

Sharding: batch B=32 -> 4 samples/core, 12 (b,c) pairs per core.

Device per (b,c), with fp8(e4m3)-quantized inputs:
    P[rb] = x8 @ y8^T - 0.5*y2[j] - 0.5*x2[i]   (fp8 DoubleRow matmuls,
            rank-1 norm terms appended as a 3-way fp8 split, K=3 matmul)
          = -0.5 * d2[i, j]   for i-block rb, all j          (PSUM f32)
    row path: pm[p, rb] = max_j P[rb]            (DVE tensor_tensor_reduce)
    col path: qm2 = max_rb P -> colv[j] = max_p qm2  (ACT cvt + DVE/Pool max,
              GPSIMD partition reduce)
Host: fp8 quantize + transposes + norms; dice term (exact f32 inputs);
final min/max/sqrt/mean combine from pm ([128,48]) and colv ([1,6144]).
"""

import numpy as np
import ml_dtypes

import concourse.bass as bass
import concourse.bacc as bacc
import concourse.mybir as mybir
import concourse.tile as tile
from concourse.bass_utils import run_bass_kernel_spmd
from bass_rust import AxisListType

B, C, H, W = 32, 3, 512, 512
NCORES = 8
BPC = B // NCORES           # samples per core
NBC = BPC * C               # (b,c) pairs per core = 12
WEIGHT_DICE = 0.4
WEIGHT_HAUSDORFF = 0.6
SMOOTH = 1e-05

f32 = mybir.dt.float32
bf16 = mybir.dt.bfloat16
f8 = mybir.dt.float8e4
ALU = mybir.AluOpType
DR = mybir.MatmulPerfMode.DoubleRow
F8NP = ml_dtypes.float8_e4m3
NEG = -3.0e38

_CACHE = {}


def _build(repeat=1):
    nc = bacc.Bacc(None)
    # x8/y8 transposed and w-interleaved: xyt[bc, w, 0] = x8^T[w], [bc, w, 1] = y8^T[w]
    xyt_d = nc.dram_tensor("xyt", [NBC, W, 2, H], f8, kind="ExternalInput")
    # norm-append operands (3-way fp8 split of -0.5*x2 / -0.5*y2):
    # aug[bc, k, 0:1024] = lhsT (rb, two, m), aug[bc, k, 1024:2048] = rhs (two, n)
    aug_d = nc.dram_tensor("aug", [NBC, 3, 2048], f8, kind="ExternalInput")
    # pm[p, 4*bc+rb] = max_j P[rb]
    # col[bc, 512*rb + j] = max_p P[rb][p, j]  (host folds rb, then min_j)
    pm_d = nc.dram_tensor("pm", [128, 4 * NBC], f32, kind="ExternalOutput")
    col_d = nc.dram_tensor("col", [NBC, 4 * H], f32, kind="ExternalOutput")

    with tile.TileContext(nc) as tc:
        with (
            tc.tile_pool(name="const", bufs=1) as cpool,
            tc.tile_pool(name="xy", bufs=3) as xypool,
            tc.tile_pool(name="aug", bufs=4) as augpool,
            tc.tile_pool(name="q", bufs=2) as qpool,
            tc.tile_pool(name="colp", bufs=6) as colpool,
            tc.tile_pool(name="psum", bufs=2, space="PSUM") as ppool,
        ):
            pmres = cpool.tile([128, 4 * NBC], f32, tag="pmres")

            pending_col = []  # (bc, coltile) awaiting shipment (lag-2)
            aug_tiles = {}  # bc -> tile, DMA-prefetched 2 iterations ahead

            def fetch_aug(b):
                if b < NBC and b not in aug_tiles:
                    t = augpool.tile([3, 2048], f8, tag="augt", name=f"augt_{b}")
                    nc.scalar.dma_start(t[:], aug_d[b])
                    aug_tiles[b] = t

            for bc in [b for _ in range(repeat) for b in range(NBC)]:
                xyts = xypool.tile([128, 2 * 4 * H], f8, tag="xyts")
                xy5 = xyts[:].rearrange("p (wb t i) -> p wb t i", wb=4, t=2)
                xyv = xyts[:].rearrange("p (wb ti) -> p wb ti", wb=4)
                xysrc = xyt_d[bc].rearrange("(wb p) t i -> p wb (t i)", wb=4)
                if bc == 0:
                    # split the first load across both queues to cut the
                    # time-to-first-matmul roughly in half
                    nc.sync.dma_start(xyv[:, 0:2, :], xysrc[:, 0:2, :])
                    nc.scalar.dma_start(xyv[:, 2:4, :], xysrc[:, 2:4, :])
                else:
                    nc.sync.dma_start(xyv, xysrc)
                # norm-append operands, prefetched 2 bcs ahead on the ACT
                # queue so appends never wait on the aug DMA
                fetch_aug(bc)
                fetch_aug(bc + 1)
                fetch_aug(bc + 2)
                augt = aug_tiles[bc]
                # ship col partials from 2 iterations ago (data long since
                # ready -> no head-of-line blocking on the ACT queue)
                if len(pending_col) >= 2:
                    pbc, ptile = pending_col.pop(0)
                    nc.scalar.dma_start(col_d[pbc : pbc + 1, :], ptile[:])
                al4 = augt[:, 0:1024].rearrange(
                    "k (rb two m) -> k rb two m", rb=4, two=2
                )
                ar = augt[:, 1024:2048].rearrange("k (two n) -> k two n", two=2)

                s01 = qpool.tile([128, 4 * 256], f32, tag="s01")
                coltile = colpool.tile([1, 4 * H], f32, tag="coltile")
                # per rb: one single-bank PSUM tile (fine-grained rotation),
                # 2 main DoubleRow matmuls + norm append, col partition-reduce
                # (Pool, the scarce engine, emitted first), then the row ttr
                for rb in range(4):
                    P = ppool.tile([128, H], f32, tag=f"P{rb}", name=f"P{rb}_{bc}")
                    for u in range(2):
                        nc.tensor.matmul(
                            P[:],
                            xy5[:, 2 * u : 2 * u + 2, 0, 128 * rb : 128 * rb + 128],
                            xy5[:, 2 * u : 2 * u + 2, 1, :],
                            start=(u == 0),
                            stop=False,
                            perf_mode=DR,
                        )
                    # norm append: half0 sums y2 splits, half1 x2 splits
                    nc.tensor.matmul(
                        P[:], al4[:, rb], ar, start=False, stop=True,
                        perf_mode=DR,
                    )
                    # col path: partition max for this tile (GPSIMD)
                    nc.gpsimd.tensor_reduce(
                        coltile[:, 512 * rb : 512 * rb + 512],
                        P[:],
                        axis=AxisListType.C,
                        op=ALU.max,
                    )
                    # row path: fused halve-max + j-reduce from PSUM (DVE)
                    nc.vector.tensor_tensor_reduce(
                        out=s01[:, 256 * rb : 256 * rb + 256],
                        in0=P[:, 0:256],
                        in1=P[:, 256:512],
                        scale=1.0,
                        scalar=NEG,
                        op0=ALU.max,
                        op1=ALU.max,
                        accum_out=pmres[:, 4 * bc + rb : 4 * bc + rb + 1],
                    )
                pending_col.append((bc, coltile))

            for pbc, ptile in pending_col:
                nc.scalar.dma_start(col_d[pbc : pbc + 1, :], ptile[:])
            nc.sync.dma_start(pm_d[:], pmres[:])
    nc.finalize()
    return nc


def _split3(v):
    """3-term fp8(e4m3) split of v: s1+s2+s3 ~= v to ~2^-12 relative."""
    s1 = v.astype(F8NP)
    r = v - s1.astype(np.float32)
    s2 = r.astype(F8NP)
    r -= s2.astype(np.float32)
    s3 = r.astype(F8NP)
    return s1, s2, s3


def kernel(input, target, _stats=None):
    x = np.asarray(input, dtype=np.float32)
    y = np.asarray(target, dtype=np.float32)

    # ---- host: dice term (exact f32 inputs) ----
    xf = x.reshape(B, -1).astype(np.float64)
    yf = y.reshape(B, -1).astype(np.float64)
    inter = (xf * yf).sum(axis=1)
    union = xf.sum(axis=1) + yf.sum(axis=1)
    dice = float(np.mean(1.0 - (2.0 * inter + SMOOTH) / (union + SMOOTH)))

    # ---- host: fp8 quantize + layout prep ----
    x8 = x.astype(F8NP)
    y8 = y.astype(F8NP)
    xq = x8.astype(np.float64)
    yq = y8.astype(np.float64)
    x2 = (xq * xq).sum(axis=-1).astype(np.float32)  # [B,C,H] norms of quantized pts
    y2 = (yq * yq).sum(axis=-1).astype(np.float32)
    xt8 = np.ascontiguousarray(x8.transpose(0, 1, 3, 2))  # [B,C,W,H] fp8
    yt8 = np.ascontiguousarray(y8.transpose(0, 1, 3, 2))

    in_maps = []
    for c in range(NCORES):
        b0 = c * BPC
        # [NBC, W, 2, H]: x^T and y^T interleaved per w row
        xyt = np.stack(
            [
                xt8[b0 : b0 + BPC].reshape(NBC, W, H),
                yt8[b0 : b0 + BPC].reshape(NBC, W, H),
            ],
            axis=2,
        )
        # lhsT[k, bc, rb, 0, :] = 1 ; [k, bc, rb, 1, :] = k-th split of -0.5*x2
        xs = _split3((-0.5 * x2[b0 : b0 + BPC]).reshape(NBC, 4, 128))
        auglhs = np.zeros((3, NBC, 4, 2, 128), dtype=F8NP)
        auglhs[:, :, :, 0, :] = np.float32(1.0)
        for k in range(3):
            auglhs[k, :, :, 1, :] = xs[k]
        # rhs[k, bc, 0, :] = k-th split of -0.5*y2 ; rhs[k, bc, 1, :] = 1
        ys = _split3((-0.5 * y2[b0 : b0 + BPC]).reshape(NBC, H))
        augrhs = np.zeros((3, NBC, 2, H), dtype=F8NP)
        for k in range(3):
            augrhs[k, :, 0, :] = ys[k]
        augrhs[:, :, 1, :] = np.float32(1.0)
        aug = np.concatenate(
            [auglhs.reshape(3, NBC, 1024), augrhs.reshape(3, NBC, 1024)], axis=2
        ).transpose(1, 0, 2)  # [NBC, 3, 2048]
        in_maps.append(
            {
                "xyt": np.ascontiguousarray(xyt),
                "aug": np.ascontiguousarray(aug),
            }
        )

    if "nc" not in _CACHE:
        _CACHE["nc"] = _build()
    nc = _CACHE["nc"]

    import time as _time

    t0 = _time.time()
    br = run_bass_kernel_spmd(nc, in_maps, list(range(NCORES)), trace=False)
    t1 = _time.time()
    if isinstance(_stats, dict):
        _stats["wall_s"] = t1 - t0
        reps = _stats.get("repeats", 0)
        times = []
        for _ in range(reps):
            ta = _time.time()
            br = run_bass_kernel_spmd(nc, in_maps, list(range(NCORES)), trace=False)
            times.append(_time.time() - ta)
        _stats["repeat_wall_s"] = times

    # ---- host: combine ----
    hds = []
    for c in range(NCORES):
        pm = np.asarray(br.results[c]["pm"], dtype=np.float32)  # [128, 48]
        colv = np.asarray(br.results[c]["col"], dtype=np.float32).reshape(
            NBC, 4, H
        )  # per-bc col partials by i-block
        for bc in range(NBC):
            dxy2 = -2.0 * float(pm[:, 4 * bc : 4 * bc + 4].min())
            dyx2 = -2.0 * float(colv[bc].max(axis=0).min())
            hds.append(np.sqrt(max(dxy2, dyx2, 0.0)))
    hd = float(np.mean(hds))

    loss = WEIGHT_DICE * dice + WEIGHT_HAUSDORFF * hd
    return np.float32(loss)


# revision 32
# speedup vs baseline: 1.3589x; 1.1057x over previous
"""Trainium2 Bass kernel for CombinedLoss (dice + hausdorff), 8-core SPMD.

Sharding: batch B=32 -> 4 samples/core, 12 (b,c) pairs per core.

Device per (b,c), with fp8(e4m3)-quantized inputs:
    P[rb] = x8 @ y8^T - 0.5*y2[j] - 0.5*x2[i]   (fp8 DoubleRow matmuls,
            rank-1 norm terms appended as a 3-way fp8 split, K=3 matmul)
          = -0.5 * d2[i, j]   for i-block rb, all j          (PSUM f32)
    row path: pm[p, rb] = max_j P[rb]            (DVE tensor_tensor_reduce)
    col path: qm2 = max_rb P -> colv[j] = max_p qm2  (ACT cvt + DVE/Pool max,
              GPSIMD partition reduce)
Host: fp8 quantize + transposes + norms; dice term (exact f32 inputs);
final min/max/sqrt/mean combine from pm ([128,48]) and colv ([1,6144]).
"""

import numpy as np
import ml_dtypes

import concourse.bass as bass
import concourse.bacc as bacc
import concourse.mybir as mybir
import concourse.tile as tile
from concourse.bass_utils import run_bass_kernel_spmd
from bass_rust import AxisListType

B, C, H, W = 32, 3, 512, 512
NCORES = 8
BPC = B // NCORES           # samples per core
NBC = BPC * C               # (b,c) pairs per core = 12
WEIGHT_DICE = 0.4
WEIGHT_HAUSDORFF = 0.6
SMOOTH = 1e-05

f32 = mybir.dt.float32
bf16 = mybir.dt.bfloat16
f8 = mybir.dt.float8e4
ALU = mybir.AluOpType
DR = mybir.MatmulPerfMode.DoubleRow
F8NP = ml_dtypes.float8_e4m3
NEG = -3.0e38

_CACHE = {}


def _build(repeat=1):
    nc = bacc.Bacc(None)
    # x8/y8 transposed and w-interleaved: xyt[bc, w, 0] = x8^T[w], [bc, w, 1] = y8^T[w]
    xyt_d = nc.dram_tensor("xyt", [NBC, W, 2, H], f8, kind="ExternalInput")
    # norm-append operands (3-way fp8 split of -0.5*x2 / -0.5*y2):
    # aug[bc, k, 0:1024] = lhsT (rb, two, m), aug[bc, k, 1024:2048] = rhs (two, n)
    aug_d = nc.dram_tensor("aug", [NBC, 3, 2048], f8, kind="ExternalInput")
    # pm[p, 4*bc+rb] = max_j P[rb]
    # col[bc, 512*rb + j] = max_p P[rb][p, j]  (host folds rb, then min_j)
    pm_d = nc.dram_tensor("pm", [128, 4 * NBC], f32, kind="ExternalOutput")
    col_d = nc.dram_tensor("col", [NBC, 4 * H], f32, kind="ExternalOutput")

    with tile.TileContext(nc) as tc:
        with (
            tc.tile_pool(name="const", bufs=1) as cpool,
            tc.tile_pool(name="xy", bufs=3) as xypool,
            tc.tile_pool(name="aug", bufs=4) as augpool,
            tc.tile_pool(name="q", bufs=2) as qpool,
            tc.tile_pool(name="colp", bufs=6) as colpool,
            tc.tile_pool(name="psum", bufs=2, space="PSUM") as ppool,
        ):
            pmres = cpool.tile([128, 4 * NBC], f32, tag="pmres")

            pending_col = []  # (bc, coltile) awaiting shipment (lag-2)
            aug_tiles = {}  # bc -> tile, DMA-prefetched 2 iterations ahead

            def fetch_aug(b):
                if b < NBC and b not in aug_tiles:
                    t = augpool.tile([3, 2048], f8, tag="augt", name=f"augt_{b}")
                    nc.scalar.dma_start(t[:], aug_d[b])
                    aug_tiles[b] = t

            for bc in [b for _ in range(repeat) for b in range(NBC)]:
                xyts = xypool.tile([128, 2 * 4 * H], f8, tag="xyts")
                xy5 = xyts[:].rearrange("p (wb t i) -> p wb t i", wb=4, t=2)
                xyv = xyts[:].rearrange("p (wb ti) -> p wb ti", wb=4)
                xysrc = xyt_d[bc].rearrange("(wb p) t i -> p wb (t i)", wb=4)
                if bc == 0:
                    # split the first load across both queues to cut the
                    # time-to-first-matmul roughly in half
                    nc.sync.dma_start(xyv[:, 0:2, :], xysrc[:, 0:2, :])
                    nc.scalar.dma_start(xyv[:, 2:4, :], xysrc[:, 2:4, :])
                else:
                    nc.sync.dma_start(xyv, xysrc)
                # norm-append operands, prefetched 2 bcs ahead on the ACT
                # queue so appends never wait on the aug DMA
                fetch_aug(bc)
                fetch_aug(bc + 1)
                fetch_aug(bc + 2)
                augt = aug_tiles[bc]
                # ship col partials from 2 iterations ago (data long since
                # ready -> no head-of-line blocking on the ACT queue)
                if len(pending_col) >= 2:
                    pbc, ptile = pending_col.pop(0)
                    nc.scalar.dma_start(col_d[pbc : pbc + 1, :], ptile[:])
                al4 = augt[:, 0:1024].rearrange(
                    "k (rb two m) -> k rb two m", rb=4, two=2
                )
                ar = augt[:, 1024:2048].rearrange("k (two n) -> k two n", two=2)

                s01 = qpool.tile([128, 4 * 256], f32, tag="s01")
                coltile = colpool.tile([1, 4 * H], f32, tag="coltile")
                # per rb: one single-bank PSUM tile (fine-grained rotation),
                # 2 main DoubleRow matmuls + norm append, col partition-reduce
                # (Pool, the scarce engine, emitted first), then the row ttr
                for rb in range(4):
                    P = ppool.tile([128, H], f32, tag=f"P{rb}", name=f"P{rb}_{bc}")
                    # norm append opens the group: it only needs the
                    # (prefetched) aug tile, so it never delays the tile
                    nc.tensor.matmul(
                        P[:], al4[:, rb], ar, start=True, stop=False,
                        perf_mode=DR,
                    )
                    for u in range(2):
                        nc.tensor.matmul(
                            P[:],
                            xy5[:, 2 * u : 2 * u + 2, 0, 128 * rb : 128 * rb + 128],
                            xy5[:, 2 * u : 2 * u + 2, 1, :],
                            start=False,
                            stop=(u == 1),
                            perf_mode=DR,
                        )
                    # col path: partition max for this tile (GPSIMD)
                    nc.gpsimd.tensor_reduce(
                        coltile[:, 512 * rb : 512 * rb + 512],
                        P[:],
                        axis=AxisListType.C,
                        op=ALU.max,
                    )
                    # row path: fused halve-max + j-reduce from PSUM (DVE)
                    nc.vector.tensor_tensor_reduce(
                        out=s01[:, 256 * rb : 256 * rb + 256],
                        in0=P[:, 0:256],
                        in1=P[:, 256:512],
                        scale=1.0,
                        scalar=NEG,
                        op0=ALU.max,
                        op1=ALU.max,
                        accum_out=pmres[:, 4 * bc + rb : 4 * bc + rb + 1],
                    )
                pending_col.append((bc, coltile))

            for pbc, ptile in pending_col:
                nc.scalar.dma_start(col_d[pbc : pbc + 1, :], ptile[:])
            nc.sync.dma_start(pm_d[:], pmres[:])
    nc.finalize()
    return nc


def _split3(v):
    """3-term fp8(e4m3) split of v: s1+s2+s3 ~= v to ~2^-12 relative."""
    s1 = v.astype(F8NP)
    r = v - s1.astype(np.float32)
    s2 = r.astype(F8NP)
    r -= s2.astype(np.float32)
    s3 = r.astype(F8NP)
    return s1, s2, s3


def kernel(input, target, _stats=None):
    x = np.asarray(input, dtype=np.float32)
    y = np.asarray(target, dtype=np.float32)

    # ---- host: dice term (exact f32 inputs) ----
    xf = x.reshape(B, -1).astype(np.float64)
    yf = y.reshape(B, -1).astype(np.float64)
    inter = (xf * yf).sum(axis=1)
    union = xf.sum(axis=1) + yf.sum(axis=1)
    dice = float(np.mean(1.0 - (2.0 * inter + SMOOTH) / (union + SMOOTH)))

    # ---- host: fp8 quantize + layout prep ----
    x8 = x.astype(F8NP)
    y8 = y.astype(F8NP)
    xq = x8.astype(np.float64)
    yq = y8.astype(np.float64)
    x2 = (xq * xq).sum(axis=-1).astype(np.float32)  # [B,C,H] norms of quantized pts
    y2 = (yq * yq).sum(axis=-1).astype(np.float32)
    xt8 = np.ascontiguousarray(x8.transpose(0, 1, 3, 2))  # [B,C,W,H] fp8
    yt8 = np.ascontiguousarray(y8.transpose(0, 1, 3, 2))

    in_maps = []
    for c in range(NCORES):
        b0 = c * BPC
        # [NBC, W, 2, H]: x^T and y^T interleaved per w row
        xyt = np.stack(
            [
                xt8[b0 : b0 + BPC].reshape(NBC, W, H),
                yt8[b0 : b0 + BPC].reshape(NBC, W, H),
            ],
            axis=2,
        )
        # lhsT[k, bc, rb, 0, :] = 1 ; [k, bc, rb, 1, :] = k-th split of -0.5*x2
        xs = _split3((-0.5 * x2[b0 : b0 + BPC]).reshape(NBC, 4, 128))
        auglhs = np.zeros((3, NBC, 4, 2, 128), dtype=F8NP)
        auglhs[:, :, :, 0, :] = np.float32(1.0)
        for k in range(3):
            auglhs[k, :, :, 1, :] = xs[k]
        # rhs[k, bc, 0, :] = k-th split of -0.5*y2 ; rhs[k, bc, 1, :] = 1
        ys = _split3((-0.5 * y2[b0 : b0 + BPC]).reshape(NBC, H))
        augrhs = np.zeros((3, NBC, 2, H), dtype=F8NP)
        for k in range(3):
            augrhs[k, :, 0, :] = ys[k]
        augrhs[:, :, 1, :] = np.float32(1.0)
        aug = np.concatenate(
            [auglhs.reshape(3, NBC, 1024), augrhs.reshape(3, NBC, 1024)], axis=2
        ).transpose(1, 0, 2)  # [NBC, 3, 2048]
        in_maps.append(
            {
                "xyt": np.ascontiguousarray(xyt),
                "aug": np.ascontiguousarray(aug),
            }
        )

    if "nc" not in _CACHE:
        _CACHE["nc"] = _build()
    nc = _CACHE["nc"]

    import time as _time

    t0 = _time.time()
    br = run_bass_kernel_spmd(nc, in_maps, list(range(NCORES)), trace=False)
    t1 = _time.time()
    if isinstance(_stats, dict):
        _stats["wall_s"] = t1 - t0
        reps = _stats.get("repeats", 0)
        times = []
        for _ in range(reps):
            ta = _time.time()
            br = run_bass_kernel_spmd(nc, in_maps, list(range(NCORES)), trace=False)
            times.append(_time.time() - ta)
        _stats["repeat_wall_s"] = times

    # ---- host: combine ----
    hds = []
    for c in range(NCORES):
        pm = np.asarray(br.results[c]["pm"], dtype=np.float32)  # [128, 48]
        colv = np.asarray(br.results[c]["col"], dtype=np.float32).reshape(
            NBC, 4, H
        )  # per-bc col partials by i-block
        for bc in range(NBC):
            dxy2 = -2.0 * float(pm[:, 4 * bc : 4 * bc + 4].min())
            dyx2 = -2.0 * float(colv[bc].max(axis=0).min())
            hds.append(np.sqrt(max(dxy2, dyx2, 0.0)))
    hd = float(np.mean(hds))

    loss = WEIGHT_DICE * dice + WEIGHT_HAUSDORFF * hd
    return np.float32(loss)


# revision 34
# speedup vs baseline: 1.4497x; 1.0668x over previous
"""Trainium2 Bass kernel for CombinedLoss (dice + hausdorff), 8-core SPMD.

Sharding: batch B=32 -> 4 samples/core, 12 (b,c) pairs per core.

Device per (b,c), with fp8(e4m3)-quantized inputs:
    P[rb] = x8 @ y8^T - 0.5*y2[j] - 0.5*x2[i]   (fp8 DoubleRow matmuls,
            rank-1 norm terms appended as a 3-way fp8 split, K=3 matmul)
          = -0.5 * d2[i, j]   for i-block rb, all j          (PSUM f32)
    row path: pm[p, rb] = max_j P[rb]            (DVE tensor_tensor_reduce)
    col path: qm2 = max_rb P -> colv[j] = max_p qm2  (ACT cvt + DVE/Pool max,
              GPSIMD partition reduce)
Host: fp8 quantize + transposes + norms; dice term (exact f32 inputs);
final min/max/sqrt/mean combine from pm ([128,48]) and colv ([1,6144]).
"""

import numpy as np
import ml_dtypes

import concourse.bass as bass
import concourse.bacc as bacc
import concourse.mybir as mybir
import concourse.tile as tile
from concourse.bass_utils import run_bass_kernel_spmd
from bass_rust import AxisListType

B, C, H, W = 32, 3, 512, 512
NCORES = 8
BPC = B // NCORES           # samples per core
NBC = BPC * C               # (b,c) pairs per core = 12
WEIGHT_DICE = 0.4
WEIGHT_HAUSDORFF = 0.6
SMOOTH = 1e-05

f32 = mybir.dt.float32
bf16 = mybir.dt.bfloat16
f8 = mybir.dt.float8e4
ALU = mybir.AluOpType
DR = mybir.MatmulPerfMode.DoubleRow
F8NP = ml_dtypes.float8_e4m3
NEG = -3.0e38

_CACHE = {}


def _build(repeat=1):
    nc = bacc.Bacc(None)
    # x8/y8 transposed and w-interleaved: xyt[bc, w, 0] = x8^T[w], [bc, w, 1] = y8^T[w]
    xyt_d = nc.dram_tensor("xyt", [NBC, W, 2, H], f8, kind="ExternalInput")
    # norm-append operands (3-way fp8 split of -0.5*x2 / -0.5*y2):
    # aug[bc, k, 0:1024] = lhsT (rb, two, m), aug[bc, k, 1024:2048] = rhs (two, n)
    aug_d = nc.dram_tensor("aug", [NBC, 3, 2048], f8, kind="ExternalInput")
    # pm[p, 4*bc+rb] = max_j P[rb]
    # col[bc, 512*rb + j] = max_p P[rb][p, j]  (host folds rb, then min_j)
    pm_d = nc.dram_tensor("pm", [128, 4 * NBC], f32, kind="ExternalOutput")
    col_d = nc.dram_tensor("col", [NBC, 4 * H], f32, kind="ExternalOutput")

    with tile.TileContext(nc) as tc:
        with (
            tc.tile_pool(name="const", bufs=1) as cpool,
            tc.tile_pool(name="xy", bufs=3) as xypool,
            tc.tile_pool(name="aug", bufs=4) as augpool,
            tc.tile_pool(name="q", bufs=2) as qpool,
            tc.tile_pool(name="colp", bufs=6) as colpool,
            tc.tile_pool(name="psum", bufs=2, space="PSUM") as ppool,
        ):
            pmres = cpool.tile([128, 4 * NBC], f32, tag="pmres")

            pending_col = []  # (bc, coltile) awaiting shipment (lag-2)
            aug_tiles = {}  # bc -> tile, DMA-prefetched 2 iterations ahead

            def fetch_aug(b):
                if b < NBC and b not in aug_tiles:
                    t = augpool.tile([3, 2048], f8, tag="augt", name=f"augt_{b}")
                    nc.scalar.dma_start(t[:], aug_d[b])
                    aug_tiles[b] = t

            for bc in [b for _ in range(repeat) for b in range(NBC)]:
                xyts = xypool.tile([128, 2 * 4 * H], f8, tag="xyts")
                xy5 = xyts[:].rearrange("p (wb t i) -> p wb t i", wb=4, t=2)
                xyv = xyts[:].rearrange("p (wb ti) -> p wb ti", wb=4)
                xysrc = xyt_d[bc].rearrange("(wb p) t i -> p wb (t i)", wb=4)
                if bc == 0:
                    # split the first load across both queues to cut the
                    # time-to-first-matmul roughly in half
                    nc.sync.dma_start(xyv[:, 0:2, :], xysrc[:, 0:2, :])
                    nc.scalar.dma_start(xyv[:, 2:4, :], xysrc[:, 2:4, :])
                else:
                    nc.sync.dma_start(xyv, xysrc)
                # norm-append operands, prefetched 2 bcs ahead on the ACT
                # queue so appends never wait on the aug DMA
                fetch_aug(bc)
                fetch_aug(bc + 1)
                fetch_aug(bc + 2)
                augt = aug_tiles[bc]
                # ship col partials from 2 iterations ago (data long since
                # ready -> no head-of-line blocking on the ACT queue)
                if len(pending_col) >= 2:
                    pbc, ptile = pending_col.pop(0)
                    nc.scalar.dma_start(col_d[pbc : pbc + 1, :], ptile[:])
                al4 = augt[:, 0:1024].rearrange(
                    "k (rb two m) -> k rb two m", rb=4, two=2
                )
                ar = augt[:, 1024:2048].rearrange("k (two n) -> k two n", two=2)

                s01 = qpool.tile([128, 4 * 256], f32, tag="s01")
                coltile = colpool.tile([1, 4 * H], f32, tag="coltile")
                # per rb: one single-bank PSUM tile (fine-grained rotation),
                # 2 main DoubleRow matmuls + norm append, col partition-reduce
                # (Pool, the scarce engine, emitted first), then the row ttr
                for rb in range(4):
                    P = ppool.tile([128, H], f32, tag=f"P{rb}", name=f"P{rb}_{bc}")
                    # norm append opens the group: it only needs the
                    # (prefetched) aug tile, so it never delays the tile.
                    # (bc0: the aug lands after the split xyt halves, so open
                    # with the mains there instead.)
                    app_first = bc != 0
                    if app_first:
                        nc.tensor.matmul(
                            P[:], al4[:, rb], ar, start=True, stop=False,
                            perf_mode=DR,
                        )
                    for u in range(2):
                        nc.tensor.matmul(
                            P[:],
                            xy5[:, 2 * u : 2 * u + 2, 0, 128 * rb : 128 * rb + 128],
                            xy5[:, 2 * u : 2 * u + 2, 1, :],
                            start=(u == 0 and not app_first),
                            stop=(u == 1 and app_first),
                            perf_mode=DR,
                        )
                    if not app_first:
                        nc.tensor.matmul(
                            P[:], al4[:, rb], ar, start=False, stop=True,
                            perf_mode=DR,
                        )
                    # col path: partition max for this tile (GPSIMD)
                    nc.gpsimd.tensor_reduce(
                        coltile[:, 512 * rb : 512 * rb + 512],
                        P[:],
                        axis=AxisListType.C,
                        op=ALU.max,
                    )
                    # row path: fused halve-max + j-reduce from PSUM (DVE)
                    nc.vector.tensor_tensor_reduce(
                        out=s01[:, 256 * rb : 256 * rb + 256],
                        in0=P[:, 0:256],
                        in1=P[:, 256:512],
                        scale=1.0,
                        scalar=NEG,
                        op0=ALU.max,
                        op1=ALU.max,
                        accum_out=pmres[:, 4 * bc + rb : 4 * bc + rb + 1],
                    )
                pending_col.append((bc, coltile))

            # final flush: spread the last col ships across both queues
            for idx, (pbc, ptile) in enumerate(pending_col):
                eng = nc.scalar if idx % 2 == 0 else nc.sync
                eng.dma_start(col_d[pbc : pbc + 1, :], ptile[:])
            nc.sync.dma_start(pm_d[:], pmres[:])
    nc.finalize()
    return nc


def _split3(v):
    """3-term fp8(e4m3) split of v: s1+s2+s3 ~= v to ~2^-12 relative."""
    s1 = v.astype(F8NP)
    r = v - s1.astype(np.float32)
    s2 = r.astype(F8NP)
    r -= s2.astype(np.float32)
    s3 = r.astype(F8NP)
    return s1, s2, s3


def kernel(input, target, _stats=None):
    x = np.asarray(input, dtype=np.float32)
    y = np.asarray(target, dtype=np.float32)

    # ---- host: dice term (exact f32 inputs) ----
    xf = x.reshape(B, -1).astype(np.float64)
    yf = y.reshape(B, -1).astype(np.float64)
    inter = (xf * yf).sum(axis=1)
    union = xf.sum(axis=1) + yf.sum(axis=1)
    dice = float(np.mean(1.0 - (2.0 * inter + SMOOTH) / (union + SMOOTH)))

    # ---- host: fp8 quantize + layout prep ----
    x8 = x.astype(F8NP)
    y8 = y.astype(F8NP)
    xq = x8.astype(np.float64)
    yq = y8.astype(np.float64)
    x2 = (xq * xq).sum(axis=-1).astype(np.float32)  # [B,C,H] norms of quantized pts
    y2 = (yq * yq).sum(axis=-1).astype(np.float32)
    xt8 = np.ascontiguousarray(x8.transpose(0, 1, 3, 2))  # [B,C,W,H] fp8
    yt8 = np.ascontiguousarray(y8.transpose(0, 1, 3, 2))

    in_maps = []
    for c in range(NCORES):
        b0 = c * BPC
        # [NBC, W, 2, H]: x^T and y^T interleaved per w row
        xyt = np.stack(
            [
                xt8[b0 : b0 + BPC].reshape(NBC, W, H),
                yt8[b0 : b0 + BPC].reshape(NBC, W, H),
            ],
            axis=2,
        )
        # lhsT[k, bc, rb, 0, :] = 1 ; [k, bc, rb, 1, :] = k-th split of -0.5*x2
        xs = _split3((-0.5 * x2[b0 : b0 + BPC]).reshape(NBC, 4, 128))
        auglhs = np.zeros((3, NBC, 4, 2, 128), dtype=F8NP)
        auglhs[:, :, :, 0, :] = np.float32(1.0)
        for k in range(3):
            auglhs[k, :, :, 1, :] = xs[k]
        # rhs[k, bc, 0, :] = k-th split of -0.5*y2 ; rhs[k, bc, 1, :] = 1
        ys = _split3((-0.5 * y2[b0 : b0 + BPC]).reshape(NBC, H))
        augrhs = np.zeros((3, NBC, 2, H), dtype=F8NP)
        for k in range(3):
            augrhs[k, :, 0, :] = ys[k]
        augrhs[:, :, 1, :] = np.float32(1.0)
        aug = np.concatenate(
            [auglhs.reshape(3, NBC, 1024), augrhs.reshape(3, NBC, 1024)], axis=2
        ).transpose(1, 0, 2)  # [NBC, 3, 2048]
        in_maps.append(
            {
                "xyt": np.ascontiguousarray(xyt),
                "aug": np.ascontiguousarray(aug),
            }
        )

    if "nc" not in _CACHE:
        _CACHE["nc"] = _build()
    nc = _CACHE["nc"]

    import time as _time

    t0 = _time.time()
    br = run_bass_kernel_spmd(nc, in_maps, list(range(NCORES)), trace=False)
    t1 = _time.time()
    if isinstance(_stats, dict):
        _stats["wall_s"] = t1 - t0
        reps = _stats.get("repeats", 0)
        times = []
        for _ in range(reps):
            ta = _time.time()
            br = run_bass_kernel_spmd(nc, in_maps, list(range(NCORES)), trace=False)
            times.append(_time.time() - ta)
        _stats["repeat_wall_s"] = times

    # ---- host: combine ----
    hds = []
    for c in range(NCORES):
        pm = np.asarray(br.results[c]["pm"], dtype=np.float32)  # [128, 48]
        colv = np.asarray(br.results[c]["col"], dtype=np.float32).reshape(
            NBC, 4, H
        )  # per-bc col partials by i-block
        for bc in range(NBC):
            dxy2 = -2.0 * float(pm[:, 4 * bc : 4 * bc + 4].min())
            dyx2 = -2.0 * float(colv[bc].max(axis=0).min())
            hds.append(np.sqrt(max(dxy2, dyx2, 0.0)))
    hd = float(np.mean(hds))

    loss = WEIGHT_DICE * dice + WEIGHT_HAUSDORFF * hd
    return np.float32(loss)
